# revision 1
# baseline (speedup 1.0000x reference)
"""Trainium2 Bass kernel for sliding-window attention layer (nn_E2ESWIGLULayer).

Sharding: DP over batch (2) x TP over head groups (4) = 8 cores.
Core c handles batch b=c//4, head group hg=c%4 (4 heads, 512 features).

Per-core program (feature-major / transposed layouts, f32r matmuls):
  Phase 1: qkv^T = w_qkv_hg @ hs_b^T  [1536 f, 2048 t], sumsq partials via
           ones-matmul, q^T/k^T spilled raw to DRAM, v PE-transposed to
           token-major and spilled.
  AllReduce [2, 2048] sumsq within each batch's 4-core group; rsqrt ->
           per-token norm factors folded into broadcast aCOS/aSIN/bCOS/bSIN.
  Phase 2: per head: RoPE+norm fused (6 DVE ops), windowed attention in s^T
           layout (s^T[k,q] tiles [128, 512]), additive -30000 masks on
           boundary tiles, exp on ACT (no max subtraction; |logit| <= ~7),
           denominator via ones-matmul, PV accumulation, divide.
  Phase 3: out^T partial [2048 o, 2048 t] = w_o_cols_hg @ attn^T, to DRAM.
Host: sum 4 partials per batch, transpose back.
"""
import os
from contextlib import ExitStack

import numpy as np

import concourse.bass as bass
import concourse.mybir as mybir
import concourse.tile as tile
from concourse import bacc
from concourse.bass_utils import run_bass_kernel_spmd
from concourse.masks import make_identity

H = 2048
NH = 16
HD = 128
WINDOW = 1024
EPS = 1e-6
THETA = 10000.0
B = 2
S = 2048
HG = 4            # head groups (TP degree)
HPG = NH // HG    # heads per group
FPG = HPG * HD    # features per group (512)
N_CORES = 8
P = 128
TC = 512          # token chunk (matmul free dim)
NTC = S // TC     # 4
NET = H // P      # 16 contraction tiles
NFT = 3 * FPG // P  # 12 output feature tiles (q 0-3, k 4-7, v 8-11)

f32 = mybir.dt.float32
f32r = mybir.dt.float32r
AF = mybir.ActivationFunctionType
ALU = mybir.AluOpType

_cache = {}


def _host_consts():
    pos = np.arange(S, dtype=np.float64)
    invf = 1.0 / (THETA ** (np.arange(0, HD, 2, dtype=np.float64) / HD))
    ang = invf[:, None] * pos[None, :]
    c64 = np.cos(ang)
    s64 = np.sin(ang)
    cos = np.concatenate([c64, c64], axis=0).astype(np.float32)   # [128, S]
    sin = np.concatenate([-s64, s64], axis=0).astype(np.float32)  # [-sin; +sin]
    # 8 partial-block mask patterns [128 k, 512 q]; deltas -384..0, 640..1024
    deltas = [-384, -256, -128, 0, 640, 768, 896, 1024]
    masks = np.zeros((8, P, TC), np.float32)
    kk = np.arange(P)[:, None]
    qq = np.arange(TC)[None, :]
    for j, d in enumerate(deltas):
        valid = (d + qq - kk >= 0) & (d + qq - kk <= WINDOW - 1)
        masks[j] = np.where(valid, 0.0, -30000.0)
    import ml_dtypes
    masks = masks.astype(ml_dtypes.bfloat16)
    return cos, sin, masks


def _kb_list(qc):
    """Valid key blocks for q-chunk qc: (kb, pattern or None, col0, col1)."""
    out = []
    for kb in range(S // P):
        d = qc * TC - kb * P
        if d < -(TC - P) or d > WINDOW:
            continue
        if P <= d <= WINDOW - TC:
            out.append((kb, None, 0, 0))
            continue
        pat = d // P + 3 if d <= 0 else d // P - 5 + 4
        if d <= 0:
            c0, c1 = 0, min(TC, P - 1 - d)
        else:
            c0, c1 = max(0, WINDOW - d), TC
        out.append((kb, pat, c0, c1))
    return out


def _build(apply_norm_w, sim_mode=False):
    key = ("nc", apply_norm_w, sim_mode)
    if key in _cache:
        return _cache[key]

    nc = bacc.Bacc("TRN2", target_bir_lowering=False, debug=False,
                   num_devices=1 if sim_mode else N_CORES)

    hsT_in = nc.dram_tensor("hsT", [H, S], f32, kind="ExternalInput").ap()
    wT_in = nc.dram_tensor("wT", [H, 3 * FPG], f32, kind="ExternalInput").ap()
    woT_in = nc.dram_tensor("woT", [FPG, H], f32, kind="ExternalInput").ap()
    qw_in = nc.dram_tensor("qw", [FPG], f32, kind="ExternalInput").ap()
    kw_in = nc.dram_tensor("kw", [FPG], f32, kind="ExternalInput").ap()
    out_ext = nc.dram_tensor("outT", [H, S], f32, kind="ExternalOutput").ap()

    cos_np, sin_np, masks_np = _host_consts()
    cos_d = nc.inline_tensor(cos_np, name="cos_c").ap()
    sin_d = nc.inline_tensor(sin_np, name="sin_c").ap()
    masks_d = nc.inline_tensor(np.ascontiguousarray(
        masks_np.transpose(1, 0, 2)), name="masks_c").ap()  # [128, 8, 512]
    ones_d = nc.inline_tensor(np.ones((P, P), np.float32), name="ones_c").ap()
    scl_d = nc.inline_tensor(
        np.full((1, P), 1.0 / np.sqrt(HD), np.float32), name="scl_c").ap()

    with tile.TileContext(nc) as tc_:
        with ExitStack() as outer:
            cpool = outer.enter_context(tc_.tile_pool(name="consts", bufs=1))
            dram = outer.enter_context(
                tc_.tile_pool(name="dram", bufs=1, space="DRAM"))

            ones_col = cpool.tile([P, 1], f32r, tag="ones_col")
            nc.sync.dma_start(ones_col[:], ones_d[:, 0:1].bitcast(f32r))
            ones_row = cpool.tile([1, P], f32r, tag="ones_row")
            nc.sync.dma_start(ones_row[:], ones_d[0:1, :].bitcast(f32r))
            scl_row = cpool.tile([1, P], f32r, tag="scl_row")
            nc.sync.dma_start(scl_row[:], scl_d[:].bitcast(f32r))
            if apply_norm_w:
                qw_sb = cpool.tile([P, HPG], f32, tag="qw")
                nc.sync.dma_start(qw_sb[:], qw_in.rearrange("(a d) -> d a", d=P))
                kw_sb = cpool.tile([P, HPG], f32, tag="kw")
                nc.sync.dma_start(kw_sb[:], kw_in.rearrange("(a d) -> d a", d=P))

            qT_d = dram.tile([FPG, S], f32r)
            kT_d = dram.tile([FPG, S], f32r)
            v_d = dram.tile([S, FPG], f32r)
            ar_in_q = dram.tile([1, S], f32)
            ar_out_q = dram.tile([1, S], f32)
            ar_in_k = dram.tile([1, S], f32)
            ar_out_k = dram.tile([1, S], f32)

            # ---------------- Phase 1: QKV GEMM ----------------
            with ExitStack() as ph1:
                wpool = ph1.enter_context(tc_.tile_pool(name="w", bufs=1))
                hspool = ph1.enter_context(tc_.tile_pool(name="hs", bufs=2))
                stg = ph1.enter_context(tc_.tile_pool(name="stg", bufs=3))
                psA = ph1.enter_context(
                    tc_.tile_pool(name="psA", bufs=3, space="PSUM"))
                psQ = ph1.enter_context(
                    tc_.tile_pool(name="psQ", bufs=1, space="PSUM"))
                psK = ph1.enter_context(
                    tc_.tile_pool(name="psK", bufs=1, space="PSUM"))
                psT = ph1.enter_context(
                    tc_.tile_pool(name="psT", bufs=2, space="PSUM"))

                ident = stg.tile([P, P], f32, tag="ident")
                make_identity(nc, ident[:])

                hsT_r = hsT_in.rearrange("(et p) t -> p et t", p=P)

                def load_hs(tci):
                    hs_sb = hspool.tile([P, NET, TC], f32r, tag="hs",
                                        name="hs%d" % tci)
                    nc.sync.dma_start(
                        hs_sb[:],
                        hsT_r[:, :, tci * TC:(tci + 1) * TC].bitcast(f32r))
                    return hs_sb

                hs_next = load_hs(0)

                w_sb = wpool.tile([P, NET, 3 * FPG], f32r, tag="w")
                wT_r = wT_in.rearrange("(et p) f -> p et f", p=P).bitcast(f32r)
                for ft in range(NFT):
                    nc.sync.dma_start(
                        w_sb[:, :, ft * P:(ft + 1) * P],
                        wT_r[:, :, ft * P:(ft + 1) * P])

                for tci in range(NTC):
                    hs_sb = hs_next
                    if tci + 1 < NTC:
                        hs_next = load_hs(tci + 1)

                    ssq_ps = {}
                    for ft in range(NFT):
                        mm_ps = psA.tile([P, TC], f32, tag="mm")
                        for et in range(NET):
                            nc.tensor.matmul(
                                mm_ps[:],
                                w_sb[:, et, ft * P:(ft + 1) * P],
                                hs_sb[:, et, :],
                                start=(et == 0), stop=(et == NET - 1))
                        if ft < 8:
                            is_q = ft < 4
                            stage = stg.tile([P, TC], f32r, tag="qk_stage")
                            nc.scalar.activation(stage[:], mm_ps[:], AF.Copy)
                            dest = qT_d if is_q else kT_d
                            fo = ft * P if is_q else (ft - 4) * P
                            nc.sync.dma_start(
                                dest[fo:fo + P, tci * TC:(tci + 1) * TC],
                                stage[:])
                            sq = stg.tile([P, TC], f32r, tag="sq")
                            nc.vector.tensor_tensor(sq[:], stage[:], stage[:],
                                                    ALU.mult)
                            kq = "q" if is_q else "k"
                            if kq not in ssq_ps:
                                ssq_ps[kq] = (psQ if is_q else psK).tile(
                                    [1, TC], f32, tag="ssq_" + kq,
                                    name="ssq_" + kq)
                            nc.tensor.matmul(
                                ssq_ps[kq][:], ones_col[:], sq[:],
                                start=(ft % 4 == 0), stop=(ft % 4 == 3))
                            if ft % 4 == 3:
                                dst = ar_in_q if is_q else ar_in_k
                                off = tci * TC
                                sst = stg.tile([1, TC], f32, tag="ssq_stage")
                                nc.vector.tensor_copy(sst[:], ssq_ps[kq][:])
                                nc.sync.dma_start(
                                    dst[0:1, off:off + TC], sst[:])
                        else:
                            vst = stg.tile([P, TC], f32, tag="v_stage")
                            nc.scalar.activation(vst[:], mm_ps[:], AF.Copy)
                            for sub in range(TC // P):
                                tr_ps = psT.tile([P, P], f32, tag="tr")
                                nc.tensor.transpose(
                                    tr_ps[:], vst[:, sub * P:(sub + 1) * P],
                                    ident[:])
                                vn = stg.tile([P, P], f32r, tag="vn")
                                nc.vector.tensor_copy(vn[:], tr_ps[:])
                                r0 = tci * TC + sub * P
                                c0 = (ft - 8) * P
                                nc.sync.dma_start(
                                    v_d[r0:r0 + P, c0:c0 + P], vn[:])

            # ---------------- AllReduce of sumsq partials ----------------
            if sim_mode:
                nc.gpsimd.dma_start(ar_out_q[:], ar_in_q[:])
                nc.gpsimd.dma_start(ar_out_k[:], ar_in_k[:])
            else:
                nc.gpsimd.collective_compute(
                    "AllReduce", ALU.add,
                    replica_groups=[[0, 1, 2, 3], [4, 5, 6, 7]],
                    ins=[ar_in_q.opt()], outs=[ar_out_q.opt()])
                nc.gpsimd.collective_compute(
                    "AllReduce", ALU.add,
                    replica_groups=[[0, 1, 2, 3], [4, 5, 6, 7]],
                    ins=[ar_in_k.opt()], outs=[ar_out_k.opt()])

            with ExitStack() as ph23:
                attn_pool = ph23.enter_context(
                    tc_.tile_pool(name="attn", bufs=1))
                attn_sb = attn_pool.tile([P, HPG, S], f32r, tag="attn")

                # ---------------- Phase 2: attention ----------------
                with ExitStack() as ph2:
                    npool = ph2.enter_context(
                        tc_.tile_pool(name="normf", bufs=1))
                    masks_sb = npool.tile([P, 8, TC], mybir.dt.bfloat16, tag="masks")
                    nc.sync.dma_start(masks_sb[:], masks_d[:])
                    acos = npool.tile([P, S], f32, tag="acos")
                    asin = npool.tile([P, S], f32, tag="asin")
                    bcos = npool.tile([P, S], f32, tag="bcos")
                    bsin = npool.tile([P, S], f32, tag="bsin")

                    # -- norm-factor table build (transient scratch) --
                    with ExitStack() as tb:
                        cspool = tb.enter_context(
                            tc_.tile_pool(name="cs", bufs=1))
                        tbp = tb.enter_context(
                            tc_.tile_pool(name="tb", bufs=2))
                        psN = tb.enter_context(
                            tc_.tile_pool(name="psN", bufs=2, space="PSUM"))
                        cos_sb = cspool.tile([P, S], f32, tag="cos")
                        nc.sync.dma_start(cos_sb[:], cos_d[:])
                        sin_sb = cspool.tile([P, S], f32, tag="sin")
                        nc.sync.dma_start(sin_sb[:], sin_d[:])
                        for tci in range(NTC):
                            sl = slice(tci * TC, (tci + 1) * TC)
                            for side in range(2):  # 0: q, 1: k
                                aro = ar_out_q if side == 0 else ar_out_k
                                off = tci * TC
                                ssqf = tbp.tile([1, TC], f32, tag="ssqf")
                                nc.sync.dma_start(
                                    ssqf[:], aro[0:1, off:off + TC])
                                var = tbp.tile([1, TC], f32, tag="var")
                                nc.vector.tensor_scalar(
                                    var[:], ssqf[:], 1.0 / H, EPS,
                                    ALU.mult, ALU.add)
                                inv = tbp.tile([1, TC], f32, tag="invr")
                                nc.vector.reciprocal(inv[:], var[:])
                                rsc = tbp.tile([1, TC], f32r, tag="rsc")
                                nc.scalar.activation(rsc[:], inv[:], AF.Sqrt)
                                lt = scl_row if side == 0 else ones_row
                                nf_ps = psN.tile([P, TC], f32, tag="nf")
                                nc.tensor.matmul(
                                    nf_ps[:], lt[:], rsc[:],
                                    start=True, stop=True)
                                ctab = acos if side == 0 else bcos
                                stab = asin if side == 0 else bsin
                                nc.vector.tensor_tensor(
                                    ctab[:, sl], nf_ps[:], cos_sb[:, sl],
                                    ALU.mult)
                                nc.vector.tensor_tensor(
                                    stab[:, sl], nf_ps[:], sin_sb[:, sl],
                                    ALU.mult)

                    # -- attention pools --
                    qkpool = ph2.enter_context(
                        tc_.tile_pool(name="qk", bufs=2))
                    swpool = ph2.enter_context(
                        tc_.tile_pool(name="sw", bufs=2))
                    vpool = ph2.enter_context(tc_.tile_pool(name="vp", bufs=2))
                    ppool = ph2.enter_context(tc_.tile_pool(name="pp", bufs=3))
                    psS = ph2.enter_context(
                        tc_.tile_pool(name="psS", bufs=2, space="PSUM"))
                    psO = ph2.enter_context(
                        tc_.tile_pool(name="psO", bufs=2, space="PSUM"))
                    psD = ph2.enter_context(
                        tc_.tile_pool(name="psD", bufs=1, space="PSUM"))
                    psB = ph2.enter_context(
                        tc_.tile_pool(name="psB", bufs=1, space="PSUM"))

                    def rope(xraw, ctab, stab, nm):
                        xsw = swpool.tile([P, S], f32r, tag="xsw",
                                          name="xsw_" + nm)
                        nc.sync.dma_start(xsw[0:64, :], xraw[64:P, :])
                        nc.sync.dma_start(xsw[64:P, :], xraw[0:64, :])
                        nc.vector.tensor_tensor(xraw[:], xraw[:], ctab[:],
                                                ALU.mult)
                        nc.vector.tensor_tensor(xsw[:], xsw[:], stab[:],
                                                ALU.mult)
                        nc.vector.tensor_tensor(xraw[:], xraw[:], xsw[:],
                                                ALU.add)
                        return xraw

                    def load_and_rope(h):
                        qraw = qkpool.tile([P, S], f32r, tag="qraw",
                                           name="qraw%d" % h)
                        nc.sync.dma_start(qraw[:],
                                          qT_d[h * P:(h + 1) * P, :])
                        kraw = qkpool.tile([P, S], f32r, tag="kraw",
                                           name="kraw%d" % h)
                        nc.sync.dma_start(kraw[:],
                                          kT_d[h * P:(h + 1) * P, :])
                        if apply_norm_w:
                            nc.vector.tensor_scalar_mul(qraw[:], qraw[:],
                                                        qw_sb[:, h:h + 1])
                            nc.vector.tensor_scalar_mul(kraw[:], kraw[:],
                                                        kw_sb[:, h:h + 1])
                        qf = rope(qraw, acos, asin, "q%d" % h)
                        kf = rope(kraw, bcos, bsin, "k%d" % h)
                        vh = vpool.tile([P, S // P, P], f32r, tag="vh",
                                        name="vh%d" % h)
                        nc.sync.dma_start(
                            vh[:],
                            v_d[:, h * P:(h + 1) * P]
                            .rearrange("(kb p) f -> p kb f", p=P))
                        return qf, kf, vh

                    nxt = load_and_rope(0)
                    for h in range(HPG):
                        qf, kf, vh = nxt
                        if h + 1 < HPG:
                            nxt = load_and_rope(h + 1)

                        for qc in range(NTC):
                            qsl = slice(qc * TC, (qc + 1) * TC)
                            blocks = _kb_list(qc)
                            out_ps = psO.tile([P, TC], f32, tag="pv")
                            den_ps = psD.tile([1, TC], f32, tag="den")
                            for i, (kb, pat, c0, c1) in enumerate(blocks):
                                s_ps = psS.tile([P, TC], f32, tag="s")
                                nc.tensor.matmul(
                                    s_ps[:], kf[:, kb * P:(kb + 1) * P],
                                    qf[:, qsl], start=True, stop=True)
                                if pat is not None:
                                    nc.vector.tensor_tensor(
                                        s_ps[:, c0:c1], s_ps[:, c0:c1],
                                        masks_sb[:, pat, c0:c1], ALU.add)
                                p_sb = ppool.tile([P, TC], f32r, tag="p")
                                nc.scalar.activation(p_sb[:], s_ps[:], AF.Exp)
                                last = (i == len(blocks) - 1)
                                nc.tensor.matmul(
                                    out_ps[:], vh[:, kb, :], p_sb[:],
                                    start=(i == 0), stop=last)
                                nc.tensor.matmul(
                                    den_ps[:], ones_col[:], p_sb[:],
                                    start=(i == 0), stop=last)
                            rec = ppool.tile([1, TC], f32r, tag="rec")
                            with nc.allow_low_precision(
                                    reason="f32r reciprocal of softmax sum"):
                                nc.vector.reciprocal(rec[:], den_ps[:])
                            rb_ps = psB.tile([P, TC], f32, tag="rb")
                            nc.tensor.matmul(rb_ps[:], ones_row[:], rec[:],
                                             start=True, stop=True)
                            rb_sb = ppool.tile([P, TC], f32, tag="rb")
                            nc.vector.tensor_copy(rb_sb[:], rb_ps[:])
                            nc.vector.tensor_tensor(
                                attn_sb[:, h, qsl], out_ps[:], rb_sb[:],
                                ALU.mult)

                    # ---- output projection (streamed wo tiles) ----
                    wopool = ph2.enter_context(tc_.tile_pool(name="wo", bufs=2))
                    ostg = ph2.enter_context(tc_.tile_pool(name="ostg", bufs=3))
                    psP = ph2.enter_context(
                        tc_.tile_pool(name="psP", bufs=2, space="PSUM"))
                    woT_r = woT_in.rearrange("(ft p) o -> p ft o", p=P)
                    for ot in range(H // P):
                        wo_t = wopool.tile([P, HPG, P], f32r, tag="wo")
                        nc.sync.dma_start(
                            wo_t[:],
                            woT_r[:, :, ot * P:(ot + 1) * P].bitcast(f32r))
                        for tci in range(NTC):
                            o_ps = psP.tile([P, TC], f32, tag="proj")
                            for ft in range(HPG):
                                nc.tensor.matmul(
                                    o_ps[:], wo_t[:, ft, :],
                                    attn_sb[:, ft, tci * TC:(tci + 1) * TC],
                                    start=(ft == 0), stop=(ft == HPG - 1))
                            ost = ostg.tile([P, TC], f32, tag="ostage")
                            nc.scalar.activation(ost[:], o_ps[:], AF.Copy)
                            nc.sync.dma_start(
                                out_ext[ot * P:(ot + 1) * P,
                                        tci * TC:(tci + 1) * TC], ost[:])

    nc.compile()
    _cache[key] = nc
    return nc


def _prep_in_maps(hidden_states, w_qkv, q_norm_w, k_norm_w, w_o):
    hs = np.ascontiguousarray(np.asarray(hidden_states, dtype=np.float32))
    wq = np.asarray(w_qkv, dtype=np.float32)
    wo = np.asarray(w_o, dtype=np.float32)
    qw = np.asarray(q_norm_w, dtype=np.float32)
    kw = np.asarray(k_norm_w, dtype=np.float32)

    hsT = [np.ascontiguousarray(hs[b].T) for b in range(B)]
    in_maps = []
    for c in range(N_CORES):
        b, hg = divmod(c, HG)
        sl = slice(hg * FPG, (hg + 1) * FPG)
        wT = np.ascontiguousarray(
            np.concatenate([wq[0 * H:][sl], wq[1 * H:][sl], wq[2 * H:][sl]],
                           axis=0).T)
        woT = np.ascontiguousarray(wo[:, sl].T)
        in_maps.append({
            "hsT": hsT[b],
            "wT": wT,
            "woT": woT,
            "qw": np.ascontiguousarray(qw[sl]),
            "kw": np.ascontiguousarray(kw[sl]),
        })
    return in_maps


def kernel(hidden_states, w_qkv, q_norm_w, k_norm_w, w_o):
    qw = np.asarray(q_norm_w, dtype=np.float32)
    kw = np.asarray(k_norm_w, dtype=np.float32)
    apply_w = not (np.allclose(qw, 1.0) and np.allclose(kw, 1.0))

    nc = _build(apply_w)
    in_maps = _prep_in_maps(hidden_states, w_qkv, q_norm_w, k_norm_w, w_o)
    res = run_bass_kernel_spmd(
        nc, in_maps, core_ids=list(range(N_CORES)),
        trace=bool(int(os.environ.get("KERNEL_TRACE", "0"))))
    _cache["last_results"] = res

    out = np.zeros((B, S, H), np.float32)
    for b in range(B):
        acc = res.results[b * HG]["outT"].astype(np.float32).copy()
        for hg in range(1, HG):
            acc += res.results[b * HG + hg]["outT"]
        out[b] = acc.T
    return out



# revision 22
# speedup vs baseline: 1.0419x; 1.0419x over previous
"""Trainium2 Bass kernel for sliding-window attention layer (nn_E2ESWIGLULayer).

Sharding: DP over batch (2) x TP over head groups (4) = 8 cores.
Core c handles batch b=c//4, head group hg=c%4 (4 heads, 512 features).

v3 (engine-balanced, emission-ordered): every engine's own DMAs serialize
with its compute in HW issue order, so DMA traffic and elementwise work are
hand-placed on the SP/ACT/DVE/Pool timelines in the order they are needed:

  Phase 1: qkv^T = w_qkv_hg @ hs_b^T. Weights stream on SP, hs on ACT
           (first chunk split for an earlier PE start), q/k staged (ACT)
           and spilled to DRAM on SP, squares for sumsq on Pool, v staged
           on DVE, PE-transposed, packed bf16 and spilled on Pool.
  Sumsq:   one combined [2, S] AllReduce (q row 0, k row 1).
  Entry:   [2, S] vectorized rsqrt chain (DVE+ACT), norm factors broadcast
           via selector-matmuls into per-side cos/sin tables (q tables on
           DVE, k tables on Pool, folding the 1/sqrt(HD) scale).
  Phase 2: per head: RoPE (q on DVE, k on Pool; swapped-half tiles loaded
           straight from the q/k spills), windowed attention in s^T layout
           ([128 k, 512 q] tiles), additive -30000 bf16 masks alternating
           DVE/Pool, exp on ACT -> bf16 (no max subtraction; |logit|<=~7),
           denominator via bf16 ones-matmul, PV accumulation, reciprocal
           broadcast by gpsimd partition_broadcast. Next head's loads and
           rope steps are paced one step per q-chunk to stay ahead of PE.
  Phase 3: out^T partial = w_o_cols_hg @ attn^T, interleaved per q-chunk
           into the last head's attention loop, outT writes on SP.
Host: sum 4 partials per batch, transpose back.
"""
import os
from contextlib import ExitStack

import numpy as np

import concourse.bass as bass
import concourse.mybir as mybir
import concourse.tile as tile
from concourse import bacc
from concourse.bass_utils import run_bass_kernel_spmd
from concourse.masks import make_identity

H = 2048
NH = 16
HD = 128
WINDOW = 1024
EPS = 1e-6
THETA = 10000.0
B = 2
S = 2048
HG = 4            # head groups (TP degree)
HPG = NH // HG    # heads per group
FPG = HPG * HD    # features per group (512)
N_CORES = 8
P = 128
TC = 512          # token chunk (matmul free dim)
NTC = S // TC     # 4
NET = H // P      # 16 contraction tiles
NFT = 3 * FPG // P  # 12 output feature tiles (q 0-3, k 4-7, v 8-11)

f32 = mybir.dt.float32
f32r = mybir.dt.float32r
bf16 = mybir.dt.bfloat16
AF = mybir.ActivationFunctionType
ALU = mybir.AluOpType

_cache = {}


def _host_consts():
    pos = np.arange(S, dtype=np.float64)
    invf = 1.0 / (THETA ** (np.arange(0, HD, 2, dtype=np.float64) / HD))
    ang = invf[:, None] * pos[None, :]
    c64 = np.cos(ang)
    s64 = np.sin(ang)
    cos = np.concatenate([c64, c64], axis=0).astype(np.float32)   # [128, S]
    sin = np.concatenate([-s64, s64], axis=0).astype(np.float32)  # [-sin; +sin]
    # 8 partial-block mask patterns [128 k, 512 q]; deltas -384..0, 640..1024
    deltas = [-384, -256, -128, 0, 640, 768, 896, 1024]
    masks = np.zeros((8, P, TC), np.float32)
    kk = np.arange(P)[:, None]
    qq = np.arange(TC)[None, :]
    for j, d in enumerate(deltas):
        valid = (d + qq - kk >= 0) & (d + qq - kk <= WINDOW - 1)
        masks[j] = np.where(valid, 1.0, 0.0)
    import ml_dtypes
    masks = masks.astype(ml_dtypes.bfloat16)
    return cos, sin, masks


def _kb_list(qc):
    """Valid key blocks for q-chunk qc: (kb, pattern or None, col0, col1)."""
    out = []
    for kb in range(S // P):
        d = qc * TC - kb * P
        if d < -(TC - P) or d > WINDOW:
            continue
        if P <= d <= WINDOW - TC:
            out.append((kb, None, 0, 0))
            continue
        pat = d // P + 3 if d <= 0 else d // P - 5 + 4
        if d <= 0:
            c0, c1 = 0, min(TC, P - 1 - d)
        else:
            c0, c1 = max(0, WINDOW - d), TC
        out.append((kb, pat, c0, c1))
    return out


def _build(apply_norm_w, sim_mode=False):
    key = ("nc", apply_norm_w, sim_mode)
    if key in _cache:
        return _cache[key]

    import ml_dtypes

    nc = bacc.Bacc("TRN2", target_bir_lowering=False, debug=False,
                   num_devices=1 if sim_mode else N_CORES)

    hsT_in = nc.dram_tensor("hsT", [H, S], bf16, kind="ExternalInput").ap()
    wT_in = nc.dram_tensor("wT", [H, 3 * FPG], bf16, kind="ExternalInput").ap()
    woT_in = nc.dram_tensor("woT", [FPG, H], bf16, kind="ExternalInput").ap()
    qw_in = nc.dram_tensor("qw", [FPG], f32, kind="ExternalInput").ap()
    kw_in = nc.dram_tensor("kw", [FPG], f32, kind="ExternalInput").ap()
    out_ext = nc.dram_tensor("outT", [H, S], f32, kind="ExternalOutput").ap()

    cos_np, sin_np, masks_np = _host_consts()
    cos_d = nc.inline_tensor(cos_np, name="cos_c").ap()
    sin_d = nc.inline_tensor(sin_np, name="sin_c").ap()
    masks_d = nc.inline_tensor(np.ascontiguousarray(
        masks_np.transpose(1, 0, 2)), name="masks_c").ap()  # [128, 8, 512]
    ones_d = nc.inline_tensor(np.ones((P, P), np.float32), name="ones_c").ap()
    onesb_d = nc.inline_tensor(np.ones((P, 1), ml_dtypes.bfloat16),
                               name="onesb_c").ap()
    # nf-matmul selectors: rows [c=2, m=128]; q folds the 1/sqrt(HD) scale
    sel_np = np.zeros((4, P), np.float32)
    sel_np[0, :] = 1.0 / np.sqrt(HD)   # selq row 0 (q sumsq row)
    sel_np[3, :] = 1.0                 # selk row 1 (k sumsq row)
    sel_d = nc.inline_tensor(sel_np, name="sel_c").ap()

    with tile.TileContext(nc) as tc_:
        with ExitStack() as outer:
            cpool = outer.enter_context(tc_.tile_pool(name="consts", bufs=1))
            dram = outer.enter_context(
                tc_.tile_pool(name="dram", bufs=1, space="DRAM"))

            ones_col = cpool.tile([P, 1], f32r, tag="ones_col")
            nc.sync.dma_start(ones_col[:], ones_d[:, 0:1].bitcast(f32r))
            onesb_col = cpool.tile([P, 1], bf16, tag="onesb_col")
            nc.sync.dma_start(onesb_col[:], onesb_d[:])
            scl_row = cpool.tile([1, P], f32r, tag="scl_row")
            nc.sync.dma_start(scl_row[:], sel_d[0:1, :].bitcast(f32r))
            if apply_norm_w:
                qw_sb = cpool.tile([P, HPG], f32, tag="qw")
                nc.sync.dma_start(qw_sb[:], qw_in.rearrange("(a d) -> d a", d=P))
                kw_sb = cpool.tile([P, HPG], f32, tag="kw")
                nc.sync.dma_start(kw_sb[:], kw_in.rearrange("(a d) -> d a", d=P))

            qT_d = dram.tile([FPG, S], f32r)
            kT_d = dram.tile([FPG, S], f32r)
            v_d = dram.tile([S, FPG], bf16)
            ar_in = [dram.tile([2, TC], f32, name="ar_in%d" % c)
                     for c in range(NTC)]
            ar_out = [dram.tile([2, TC], f32, name="ar_out%d" % c)
                      for c in range(NTC)]

            # scratch shared by phase-1 sumsq chain and phase-2 tables
            tbp = outer.enter_context(tc_.tile_pool(name="tb", bufs=2))
            rscp = outer.enter_context(tc_.tile_pool(name="rsc", bufs=4))
            cscr = outer.enter_context(tc_.tile_pool(name="cs", bufs=2))
            nfc = []   # per-chunk rsqrt results [2, TC] (f32, bitcast later)
            coss = []  # per-chunk (cos, sin) sbuf chunks

            # ---------------- Phase 1: QKV GEMM ----------------
            with ExitStack() as ph1:
                wpool = ph1.enter_context(tc_.tile_pool(name="w", bufs=1))
                hspool = ph1.enter_context(tc_.tile_pool(name="hs", bufs=2))
                stg = ph1.enter_context(tc_.tile_pool(name="stg", bufs=2))
                qkst = ph1.enter_context(tc_.tile_pool(name="qkst", bufs=3))
                sstp = ph1.enter_context(tc_.tile_pool(name="sst", bufs=1))
                vstg = ph1.enter_context(tc_.tile_pool(name="vstg", bufs=1))
                psA = ph1.enter_context(
                    tc_.tile_pool(name="psA", bufs=4, space="PSUM"))
                psQ = ph1.enter_context(
                    tc_.tile_pool(name="psQ", bufs=1, space="PSUM"))
                psK = ph1.enter_context(
                    tc_.tile_pool(name="psK", bufs=1, space="PSUM"))
                psT = ph1.enter_context(
                    tc_.tile_pool(name="psT", bufs=1, space="PSUM"))

                ident = cpool.tile([P, P], f32, tag="ident")
                make_identity(nc, ident[:])

                # weights stream on the SP ring, issued first (ft pairs
                # keep 512B contiguous runs for full DMA rate)
                w_sb = wpool.tile([P, NET, 3 * FPG], bf16, tag="w")
                wT_r = wT_in.rearrange("(et p) f -> p et f", p=P)
                for fp in range(NFT // 2):
                    nc.sync.dma_start(
                        w_sb[:, :, fp * 2 * P:(fp + 1) * 2 * P],
                        wT_r[:, :, fp * 2 * P:(fp + 1) * 2 * P])

                hsT_r = hsT_in.rearrange("(et p) t -> p et t", p=P)

                def load_hs(tci, split=1):
                    hs_sb = hspool.tile([P, NET, TC], bf16, tag="hs",
                                        name="hs%d" % tci)
                    step = NET // split
                    for c in range(split):
                        e0 = c * step
                        nc.scalar.dma_start(
                            hs_sb[:, e0:e0 + step, :],
                            hsT_r[:, e0:e0 + step,
                                  tci * TC:(tci + 1) * TC])
                    return hs_sb

                hs_next = load_hs(0, split=4)

                for tci in range(NTC):
                    hs_sb = hs_next
                    if tci + 1 < NTC:
                        hs_next = load_hs(tci + 1)

                    csl = slice(tci * TC, (tci + 1) * TC)
                    cchunk = cscr.tile([P, TC], f32, tag="cosc",
                                       name="cosc%d" % tci)
                    nc.gpsimd.dma_start(cchunk[:], cos_d[:, csl])
                    schunk = cscr.tile([P, TC], f32, tag="sinc",
                                       name="sinc%d" % tci)
                    nc.gpsimd.dma_start(schunk[:], sin_d[:, csl])
                    coss.append((cchunk, schunk))

                    ssq_ps = {}
                    for ft in range(NFT):
                        mm_ps = psA.tile([P, TC], f32, tag="mm")
                        for et in range(NET):
                            nc.tensor.matmul(
                                mm_ps[:],
                                w_sb[:, et, ft * P:(ft + 1) * P],
                                hs_sb[:, et, :],
                                start=(et == 0), stop=(et == NET - 1))
                        if ft < 8:
                            is_q = ft < 4
                            stage = qkst.tile([P, TC], f32r, tag="qk_stage")
                            nc.vector.tensor_copy(stage[:], mm_ps[:])
                            dest = qT_d if is_q else kT_d
                            fo = ft * P if is_q else (ft - 4) * P
                            nc.gpsimd.dma_start(
                                dest[fo:fo + P, tci * TC:(tci + 1) * TC],
                                stage[:])
                            sq = stg.tile([P, TC], f32r, tag="sq")
                            nc.gpsimd.tensor_tensor(sq[:], stage[:], stage[:],
                                                    ALU.mult)
                            kq = "q" if is_q else "k"
                            if kq not in ssq_ps:
                                ssq_ps[kq] = (psQ if is_q else psK).tile(
                                    [1, TC], f32, tag="ssq_" + kq,
                                    name="ssq_" + kq)
                            nc.tensor.matmul(
                                ssq_ps[kq][:], ones_col[:], sq[:],
                                start=(ft % 4 == 0), stop=(ft % 4 == 3))
                            if ft % 4 == 3:
                                row = 0 if is_q else 1
                                off = tci * TC
                                sst = sstp.tile([1, TC], f32,
                                                tag="ssq_stage")
                                nc.vector.tensor_copy(sst[:], ssq_ps[kq][:])
                                nc.sync.dma_start(
                                    ar_in[tci][row:row + 1, :], sst[:])
                                if not is_q:
                                    # both rows of this chunk written: reduce
                                    # it and run the rsqrt chain now
                                    if sim_mode:
                                        nc.gpsimd.dma_start(
                                            ar_out[tci][:], ar_in[tci][:])
                                    else:
                                        nc.gpsimd.collective_compute(
                                            "AllReduce", ALU.add,
                                            replica_groups=[[0, 1, 2, 3],
                                                            [4, 5, 6, 7]],
                                            ins=[ar_in[tci].opt()],
                                            outs=[ar_out[tci].opt()])
                                    rr = []
                                    for side in range(2):
                                        ssqf = tbp.tile(
                                            [1, TC], f32,
                                            tag="ssqf%d" % side,
                                            name="ssqf%d_%d" % (side, tci))
                                        nc.scalar.dma_start(
                                            ssqf[:],
                                            ar_out[tci][side:side + 1, :])
                                        nc.vector.tensor_scalar(
                                            ssqf[:], ssqf[:], 1.0 / H, EPS,
                                            ALU.mult, ALU.add)
                                        nc.vector.reciprocal(ssqf[:], ssqf[:])
                                        rsc = rscp.tile(
                                            [1, TC], f32r, tag="rsc%d" % side,
                                            name="rsc%d_%d" % (side, tci))
                                        nc.scalar.activation(rsc[:], ssqf[:],
                                                             AF.Sqrt)
                                        rr.append(rsc)
                                    nfc.append(rr)
                                    if tci == NTC - 1:
                                        # preload the exp act-func set
                                        dmy = tbp.tile([1, 16], f32,
                                                       tag="dmy")
                                        nc.scalar.activation(
                                            dmy[:], rr[0][0:1, 0:16], AF.Exp)
                        else:
                            vst = stg.tile([P, TC], f32, tag="v_stage")
                            nc.vector.tensor_copy(vst[:], mm_ps[:])
                            vn4 = vstg.tile([P, TC // P, P], bf16, tag="vn4")
                            for sub in range(TC // P):
                                tr_ps = psT.tile([P, P], f32, tag="tr")
                                nc.tensor.transpose(
                                    tr_ps[:], vst[:, sub * P:(sub + 1) * P],
                                    ident[:])
                                nc.vector.tensor_copy(vn4[:, sub, :], tr_ps[:])
                            r0 = tci * TC
                            c0 = (ft - 8) * P
                            nc.gpsimd.dma_start(
                                v_d[r0:r0 + TC, c0:c0 + P]
                                .rearrange("(s p) f -> p s f", p=P), vn4[:])

                            if ft % 4 == 3:
                                row = 0 if is_q else 1
                                off = tci * TC
                                sst = sstp.tile([1, TC], f32,
                                                tag="ssq_stage")
                                nc.vector.tensor_copy(sst[:], ssq_ps[kq][:])
                                nc.sync.dma_start(
                                    ar_in[tci][row:row + 1, :], sst[:])
                                if not is_q:
                                    # both rows of this chunk written: reduce
                                    # it and run the rsqrt chain now
                                    if sim_mode:
                                        nc.gpsimd.dma_start(
                                            ar_out[tci][:], ar_in[tci][:])
                                    else:
                                        nc.gpsimd.collective_compute(
                                            "AllReduce", ALU.add,
                                            replica_groups=[[0, 1, 2, 3],
                                                            [4, 5, 6, 7]],
                                            ins=[ar_in[tci].opt()],
                                            outs=[ar_out[tci].opt()])
                                    rr = []
                                    for side in range(2):
                                        ssqf = tbp.tile(
                                            [1, TC], f32,
                                            tag="ssqf%d" % side,
                                            name="ssqf%d_%d" % (side, tci))
                                        nc.scalar.dma_start(
                                            ssqf[:],
                                            ar_out[tci][side:side + 1, :])
                                        nc.vector.tensor_scalar(
                                            ssqf[:], ssqf[:], 1.0 / H, EPS,
                                            ALU.mult, ALU.add)
                                        nc.vector.reciprocal(ssqf[:], ssqf[:])
                                        rsc = rscp.tile(
                                            [1, TC], f32r, tag="rsc%d" % side,
                                            name="rsc%d_%d" % (side, tci))
                                        nc.scalar.activation(rsc[:], ssqf[:],
                                                             AF.Sqrt)
                                        rr.append(rsc)
                                    nfc.append(rr)
                                    if tci == NTC - 1:
                                        # preload the exp act-func set
                                        dmy = tbp.tile([1, 16], f32,
                                                       tag="dmy")
                                        nc.scalar.activation(
                                            dmy[:], rr[0][0:1, 0:16], AF.Exp)
            with ExitStack() as ph23:
                attn_pool = ph23.enter_context(
                    tc_.tile_pool(name="attn", bufs=1))
                attn_sb = attn_pool.tile([P, HPG, S], bf16, tag="attn")
                npool = ph23.enter_context(tc_.tile_pool(name="normf", bufs=1))
                acos = npool.tile([P, S], f32, tag="acos")
                asin = npool.tile([P, S], f32, tag="asin")
                bcos = npool.tile([P, S], f32, tag="bcos")
                bsin = npool.tile([P, S], f32, tag="bsin")
                masks_sb = npool.tile([P, 8, TC], bf16, tag="masks")

                with ExitStack() as ph2:
                    qkpool = ph2.enter_context(
                        tc_.tile_pool(name="qk", bufs=2))
                    swpool = ph2.enter_context(
                        tc_.tile_pool(name="sw", bufs=2))
                    vpool = ph2.enter_context(tc_.tile_pool(name="vp", bufs=2))
                    ppool = ph2.enter_context(tc_.tile_pool(name="pp", bufs=3))
                    wopool = ph2.enter_context(tc_.tile_pool(name="wo", bufs=1))
                    ostg = ph2.enter_context(tc_.tile_pool(name="ostg", bufs=3))

                    tiles = {}  # h -> dict(q, qsw, k, ksw, vh)

                    def emit_qload(h, eng, chunk=None):
                        t = tiles.setdefault(h, {})
                        if "q" not in t:
                            t["q"] = qkpool.tile([P, S], f32r, tag="qraw",
                                                 name="qraw%d" % h)
                            t["qsw"] = swpool.tile([P, S], f32r, tag="qsw",
                                                   name="qsw%d" % h)
                        sl = slice(0, S) if chunk is None else \
                            slice(chunk * TC, (chunk + 1) * TC)
                        eng.dma_start(t["q"][:, sl],
                                      qT_d[h * P:(h + 1) * P, sl])
                        eng.dma_start(t["qsw"][0:64, sl],
                                      qT_d[h * P + 64:(h + 1) * P, sl])
                        eng.dma_start(t["qsw"][64:P, sl],
                                      qT_d[h * P:h * P + 64, sl])

                    def emit_kload(h, eng, chunk=None):
                        t = tiles.setdefault(h, {})
                        if "k" not in t:
                            t["k"] = qkpool.tile([P, S], f32r, tag="kraw",
                                                 name="kraw%d" % h)
                            t["ksw"] = swpool.tile([P, S], f32r, tag="ksw",
                                                   name="ksw%d" % h)
                        sl = slice(0, S) if chunk is None else \
                            slice(chunk * TC, (chunk + 1) * TC)
                        eng.dma_start(t["k"][:, sl],
                                      kT_d[h * P:(h + 1) * P, sl])
                        eng.dma_start(t["ksw"][0:64, sl],
                                      kT_d[h * P + 64:(h + 1) * P, sl])
                        eng.dma_start(t["ksw"][64:P, sl],
                                      kT_d[h * P:h * P + 64, sl])

                    def emit_vh(h):
                        t = tiles.setdefault(h, {})
                        t["vh"] = vpool.tile([P, S // P, P], bf16, tag="vh",
                                             name="vh%d" % h)
                        nc.gpsimd.dma_start(
                            t["vh"][:],
                            v_d[:, h * P:(h + 1) * P]
                            .rearrange("(kb p) f -> p kb f", p=P))

                    def emit_norm_w(h):
                        t = tiles[h]
                        nc.vector.tensor_scalar_mul(
                            t["q"][:], t["q"][:], qw_sb[:, h:h + 1])
                        nc.vector.tensor_scalar_mul(
                            t["qsw"][:], t["qsw"][:], qw_sb[:, h:h + 1])
                        nc.gpsimd.tensor_scalar_mul(
                            t["k"][:], t["k"][:], kw_sb[:, h:h + 1])
                        nc.gpsimd.tensor_scalar_mul(
                            t["ksw"][:], t["ksw"][:], kw_sb[:, h:h + 1])

                    def emit_rope_step(h, step, chunk=None):
                        t = tiles[h]
                        sl = slice(0, S) if chunk is None else \
                            slice(chunk * TC, (chunk + 1) * TC)
                        if step == 0 and apply_norm_w and chunk in (None, 0):
                            emit_norm_w(h)
                        if step == 0:
                            nc.vector.tensor_tensor(
                                t["q"][:, sl], t["q"][:, sl], acos[:, sl],
                                ALU.mult)
                            nc.gpsimd.tensor_tensor(
                                t["k"][:, sl], t["k"][:, sl], bcos[:, sl],
                                ALU.mult)
                        elif step == 1:
                            nc.vector.tensor_tensor(
                                t["qsw"][:, sl], t["qsw"][:, sl],
                                asin[:, sl], ALU.mult)
                            nc.gpsimd.tensor_tensor(
                                t["ksw"][:, sl], t["ksw"][:, sl],
                                bsin[:, sl], ALU.mult)
                        else:
                            nc.vector.tensor_tensor(
                                t["q"][:, sl], t["q"][:, sl], t["qsw"][:, sl],
                                ALU.add)
                            nc.gpsimd.tensor_tensor(
                                t["k"][:, sl], t["k"][:, sl], t["ksw"][:, sl],
                                ALU.add)

                    # ---- entry: prefetch + table build ----
                    # nf chunks (nfc) and cos/sin chunks (coss) were computed
                    # during phase 1; only the broadcast TTs remain here.
                    with ExitStack() as tb:
                        psN = tb.enter_context(
                            tc_.tile_pool(name="psN", bufs=2, space="PSUM"))

                        nc.gpsimd.dma_start(masks_sb[:], masks_d[:])
                        emit_vh(0)

                        for tci in range(NTC):
                            sl = slice(tci * TC, (tci + 1) * TC)
                            emit_qload(0, nc.sync, tci)
                            emit_kload(0, nc.sync, tci)
                            emit_kload(1, nc.gpsimd, tci)
                            if tci == 1:
                                emit_vh(1)
                            rsq, rsk = nfc[tci]
                            cchunk, schunk = coss[tci]
                            nfq_ps = psN.tile([P, TC], f32, tag="nfq")
                            nc.tensor.matmul(nfq_ps[:], scl_row[:], rsq[:],
                                             start=True, stop=True)
                            nc.vector.tensor_tensor(
                                acos[:, sl], nfq_ps[:], cchunk[:], ALU.mult)
                            nc.vector.tensor_tensor(
                                asin[:, sl], nfq_ps[:], schunk[:], ALU.mult)
                            nfk_b = tbp.tile([P, TC], f32r, tag="nfkb",
                                             name="nfkb%d" % tci)
                            nc.gpsimd.partition_broadcast(nfk_b[:], rsk[:])
                            nc.gpsimd.tensor_tensor(
                                bcos[:, sl], nfk_b[:], cchunk[:], ALU.mult)
                            nc.gpsimd.tensor_tensor(
                                bsin[:, sl], nfk_b[:], schunk[:], ALU.mult)
                            # head 0 rope for this chunk right away
                            for st in range(3):
                                emit_rope_step(0, st, tci)
                        for tci in range(NTC):
                            emit_qload(1, nc.sync, tci)

                    psS = ph2.enter_context(
                        tc_.tile_pool(name="psS", bufs=3, space="PSUM"))
                    psO = ph2.enter_context(
                        tc_.tile_pool(name="psO", bufs=2, space="PSUM"))
                    psD = ph2.enter_context(
                        tc_.tile_pool(name="psD", bufs=1, space="PSUM"))
                    psP = ph2.enter_context(
                        tc_.tile_pool(name="psP", bufs=2, space="PSUM"))

                    # ---- attention + interleaved output projection ----
                    woT_r = woT_in.rearrange("(ft p) o -> p ft o", p=P)
                    wo_sb = wopool.tile([P, HPG, H], bf16, tag="wo")

                    for h in range(HPG):
                        t = tiles[h]
                        qf, kf, vh = t["q"], t["k"], t["vh"]
                        if h + 2 < HPG:
                            emit_kload(h + 2, nc.sync)
                            emit_qload(h + 2, nc.sync)
                            emit_vh(h + 2)
                        if h == 1:
                            for ft in range(HPG):
                                nc.gpsimd.dma_start(
                                    wo_sb[:, ft, :], woT_r[:, ft, :])

                        for qc in range(NTC):
                            qsl = slice(qc * TC, (qc + 1) * TC)
                            blocks = _kb_list(qc)
                            out_ps = psO.tile([P, TC], f32, tag="pv")
                            den_ps = psD.tile([1, TC], f32, tag="den")
                            for i, (kb, pat, c0, c1) in enumerate(blocks):
                                s_ps = psS.tile([P, TC], f32, tag="s")
                                nc.tensor.matmul(
                                    s_ps[:], kf[:, kb * P:(kb + 1) * P],
                                    qf[:, qsl], start=True, stop=True)
                                p_sb = ppool.tile([P, TC], bf16, tag="p")
                                nc.scalar.activation(p_sb[:], s_ps[:], AF.Exp)
                                if pat is not None:
                                    eng = nc.vector if i % 2 == 0 else \
                                        nc.gpsimd
                                    eng.tensor_tensor(
                                        p_sb[:, c0:c1], p_sb[:, c0:c1],
                                        masks_sb[:, pat, c0:c1], ALU.mult)
                                last = (i == len(blocks) - 1)
                                nc.tensor.matmul(
                                    out_ps[:], vh[:, kb, :], p_sb[:],
                                    start=(i == 0), stop=last)
                                nc.tensor.matmul(
                                    den_ps[:], onesb_col[:], p_sb[:],
                                    start=(i == 0), stop=last)
                            rec = ppool.tile([1, TC], f32r, tag="rec")
                            with nc.allow_low_precision(
                                    reason="f32r reciprocal of softmax sum"):
                                nc.vector.reciprocal(rec[:], den_ps[:])
                            rb_sb = ppool.tile([P, TC], f32r, tag="rb")
                            nc.gpsimd.partition_broadcast(rb_sb[:], rec[:])
                            nc.vector.tensor_tensor(
                                attn_sb[:, h, qsl], out_ps[:], rb_sb[:],
                                ALU.mult)

                            # pace next head's rope
                            if h == 0:
                                for st in range(3):
                                    emit_rope_step(1, st, qc)
                            elif h + 1 < HPG and qc <= 2:
                                emit_rope_step(h + 1, qc)

                            # ---- interleaved phase 3 for this tci ----
                            if h == HPG - 1:
                                for ot in range(H // P):
                                    o_ps = psP.tile([P, TC], f32, tag="proj")
                                    for ft in range(HPG):
                                        nc.tensor.matmul(
                                            o_ps[:],
                                            wo_sb[:, ft, ot * P:(ot + 1) * P],
                                            attn_sb[:, ft, qsl],
                                            start=(ft == 0),
                                            stop=(ft == HPG - 1))
                                    ost = ostg.tile([P, TC], f32, tag="ostage")
                                    if ot % 2 == 0:
                                        nc.vector.tensor_copy(ost[:], o_ps[:])
                                    else:
                                        nc.scalar.activation(ost[:], o_ps[:],
                                                             AF.Copy)
                                    nc.sync.dma_start(
                                        out_ext[ot * P:(ot + 1) * P, qsl],
                                        ost[:])

    nc.compile()
    _cache[key] = nc
    return nc


def _prep_in_maps(hidden_states, w_qkv, q_norm_w, k_norm_w, w_o):
    hs = np.ascontiguousarray(np.asarray(hidden_states, dtype=np.float32))
    wq = np.asarray(w_qkv, dtype=np.float32)
    wo = np.asarray(w_o, dtype=np.float32)
    qw = np.asarray(q_norm_w, dtype=np.float32)
    kw = np.asarray(k_norm_w, dtype=np.float32)

    hsT = [np.ascontiguousarray(hs[b].T) for b in range(B)]
    in_maps = []
    for c in range(N_CORES):
        b, hg = divmod(c, HG)
        sl = slice(hg * FPG, (hg + 1) * FPG)
        wT = np.ascontiguousarray(
            np.concatenate([wq[0 * H:][sl], wq[1 * H:][sl], wq[2 * H:][sl]],
                           axis=0).T)
        import ml_dtypes
        woT = np.ascontiguousarray(wo[:, sl].T).astype(ml_dtypes.bfloat16)
        wT = wT.astype(ml_dtypes.bfloat16)
        in_maps.append({
            "hsT": hsT[b].astype(ml_dtypes.bfloat16),
            "wT": wT,
            "woT": woT,
            "qw": np.ascontiguousarray(qw[sl]),
            "kw": np.ascontiguousarray(kw[sl]),
        })
    return in_maps


def kernel(hidden_states, w_qkv, q_norm_w, k_norm_w, w_o):
    qw = np.asarray(q_norm_w, dtype=np.float32)
    kw = np.asarray(k_norm_w, dtype=np.float32)
    apply_w = not (np.allclose(qw, 1.0) and np.allclose(kw, 1.0))

    nc = _build(apply_w)
    in_maps = _prep_in_maps(hidden_states, w_qkv, q_norm_w, k_norm_w, w_o)
    res = run_bass_kernel_spmd(
        nc, in_maps, core_ids=list(range(N_CORES)),
        trace=bool(int(os.environ.get("KERNEL_TRACE", "0"))))
    _cache["last_results"] = res

    out = np.zeros((B, S, H), np.float32)
    for b in range(B):
        acc = res.results[b * HG]["outT"].astype(np.float32).copy()
        for hg in range(1, HG):
            acc += res.results[b * HG + hg]["outT"]
        out[b] = acc.T
    return out
